# revision 13
# baseline (speedup 1.0000x reference)
"""Trainium2 Bass kernel: 2-layer heterogeneous GCN + document aggregation,
8-core SPMD (dst-node sharding, AllGather of node tables, gather/one-hot-matmul spmm).

Self-contained: hardcodes all shapes. kernel(**inputs) -> (doc, doc_svd).
"""

from contextlib import ExitStack

import numpy as np

import concourse.bacc as bacc
import concourse.mybir as mybir
from concourse import library_config
from concourse.tile import TileContext
from concourse.bass_utils import run_bass_kernel_spmd

F32 = mybir.dt.float32
I16 = mybir.dt.int16
AF = mybir.ActivationFunctionType
ALU = mybir.AluOpType

P = 128
NC = 8
IDX_BATCH = 1024  # gather rows per dma_gather call (HW ucode limit: 2048 crashes)
EPS = 1e-9

# problem sizes
N0, N1, N2 = 10000, 50000, 30000
D, O, DW = 256, 128, 128

S0, S0P = N0 // NC, 1280   # doc shard: 1250 real rows, padded to 10 tiles
S1, S1P = N1 // NC, 6272   # type-1 node shard: 6250 real, 49 tiles
S2, S2P = N2 // NC, 3840   # type-2 node shard: 3750 real, 30 tiles
T0, T1, T2 = S0P // P, S1P // P, S2P // P
HALF1 = 4 * S1P            # 25088: int16-safe half boundary of padded N1 tables


def _ceil(a, b):
    return -(-a // b)


# ---------------------------------------------------------------------------
# Host-side edge preprocessing
# ---------------------------------------------------------------------------

class EdgePlan:
    """Static (core-independent) schedule + per-core data arrays for one edge set."""

    def __init__(self, src, dst, w, sd_real, sd_pad, ss_real, ss_pad, half_bound):
        src = np.asarray(src).astype(np.int64)
        dst = np.asarray(dst).astype(np.int64)
        w = np.asarray(w, np.float32)
        E = len(src)
        n_tiles = sd_pad // P
        core = dst // sd_real
        dloc = dst - core * sd_real
        tile_id = dloc // P
        dst_rel = (dloc % P).astype(np.float32)
        spad = (src // ss_real) * ss_pad + (src % ss_real)
        if half_bound:
            half = (spad >= half_bound).astype(np.int64)
            idx_rel = (spad - half * half_bound).astype(np.int64)
            n_halves = 2
        else:
            half = np.zeros(E, np.int64)
            idx_rel = spad
            n_halves = 1
        assert idx_rel.max() < 32768
        key = (core * n_halves + half) * n_tiles + tile_id
        counts = np.bincount(key, minlength=NC * n_halves * n_tiles).reshape(
            NC, n_halves, n_tiles
        )
        n_chunks = -(-counts // P)  # ceil
        n_chunks = n_chunks.max(axis=0)  # [n_halves, n_tiles] cross-core max
        flat = n_chunks.reshape(-1)
        coff = np.concatenate([[0], np.cumsum(flat)[:-1]]).reshape(n_halves, n_tiles)
        total_chunks = int(flat.sum())

        order = np.argsort(key, kind="stable")
        key_s = key[order]
        grp_first = np.searchsorted(key_s, np.arange(NC * n_halves * n_tiles), "left")
        pos_in_grp = np.arange(E) - grp_first[key_s]
        spos = coff[half[order], tile_id[order]] * P + pos_in_grp

        idx_flat = np.zeros((NC, total_chunks * P), np.int16)
        w_flat = np.zeros((NC, total_chunks * P), np.float32)
        rel_flat = np.zeros((NC, total_chunks * P), np.float32)
        c_s = core[order]
        idx_flat[c_s, spos] = idx_rel[order].astype(np.int16)
        w_flat[c_s, spos] = w[order]
        rel_flat[c_s, spos] = dst_rel[order]

        self.n_halves = n_halves
        self.n_tiles = n_tiles
        self.n_chunks = n_chunks  # [h][t]
        self.total_chunks = total_chunks
        self.half_nchunks = [int(n_chunks[h].sum()) for h in range(n_halves)]
        self.half_chunk_base = np.concatenate([[0], np.cumsum(self.half_nchunks)])
        # per-chunk tile schedule per half: list of (tile, first, last)
        self.sched = []
        for h in range(n_halves):
            s = []
            for t in range(n_tiles):
                nct = int(n_chunks[h][t])
                for k in range(nct):
                    s.append((t, k == 0, k == nct - 1))
            self.sched.append(s)
        # per-core wrapped arrays
        self.idx_wrapped = []  # [core][half] -> [128, Lh//16] int16
        self.w_wrapped = np.zeros((NC, P, total_chunks), np.float32)
        self.rel_wrapped = np.zeros((NC, P, total_chunks), np.float32)
        for c in range(NC):
            per_half = []
            for h in range(n_halves):
                lo = int(self.half_chunk_base[h]) * P
                hi = int(self.half_chunk_base[h + 1]) * P
                seg = idx_flat[c, lo:hi]
                if hi > lo:
                    wr = np.tile(seg.reshape(-1, 16).T, (8, 1))  # [128, Lh/16]
                else:
                    wr = np.zeros((P, 0), np.int16)
                per_half.append(np.ascontiguousarray(wr))
            self.idx_wrapped.append(per_half)
            self.w_wrapped[c] = w_flat[c].reshape(total_chunks, P).T
            self.rel_wrapped[c] = rel_flat[c].reshape(total_chunks, P).T


# ---------------------------------------------------------------------------
# Device program pieces
# ---------------------------------------------------------------------------

def _dense_layer(nc, sb, psp, xT_in, n_node_tiles, wk0, wk1, b_row, ones, outb):
    """outb[:, m, :] = relu(xT[:, m-tile].T @ W + b) over both K halves."""
    xa0 = sb.tile([P, n_node_tiles * P], F32, tag="xa0")
    xa1 = sb.tile([P, n_node_tiles * P], F32, tag="xa1")
    nc.sync.dma_start(xa0[:], xT_in[0:P, :])
    nc.sync.dma_start(xa1[:], xT_in[P : 2 * P, :])
    for m in range(n_node_tiles):
        ps = psp.tile([P, P], F32, tag="psA")
        sl = slice(m * P, (m + 1) * P)
        nc.tensor.matmul(ps[:], lhsT=xa0[:, sl], rhs=wk0[:], start=True, stop=False)
        nc.tensor.matmul(ps[:], lhsT=xa1[:, sl], rhs=wk1[:], start=False, stop=False)
        nc.tensor.matmul(ps[:], lhsT=ones[:1, :], rhs=b_row[:1, :], start=False, stop=True)
        nc.scalar.activation(outb[:, m, :], ps[:], AF.Relu)


def _spmm_T(nc, sb, gpool, spool, psp, plan, tables, idx_tiles, w_col, rel_col, iota, accT):
    """Transposed spmm accumulate: accT[:, t*128:(t+1)*128] = sum over chunks
    G_chunk.T @ S_chunk  (feat on partitions, dst on free)."""
    written = [False] * plan.n_tiles
    gchunk = 0
    for h in range(plan.n_halves):
        nch_h = plan.half_nchunks[h]
        if nch_h == 0:
            continue
        Lh = nch_h * P
        sched = plan.sched[h]
        ci = 0
        psum = None
        for b0 in range(0, Lh, IDX_BATCH):
            nidx = min(IDX_BATCH, Lh - b0)
            nch = nidx // P
            gt = gpool.tile([P, IDX_BATCH // P, P], F32, tag="gT")
            nc.gpsimd.dma_gather(
                gt[:, :nch, :], tables[h], idx_tiles[h][:, b0 // 16 : (b0 + nidx) // 16],
                nidx, nidx, P,
            )
            for k in range(nch):
                t, first, last = sched[ci]
                if first:
                    psum = psp.tile([P, P], F32, tag="psT")
                S = spool.tile([P, P], F32, tag="S")
                nc.vector.tensor_scalar(
                    S[:], iota[:], rel_col[:, gchunk : gchunk + 1],
                    w_col[:, gchunk : gchunk + 1], ALU.is_equal, ALU.mult,
                )
                nc.tensor.matmul(psum[:], lhsT=gt[:, k, :], rhs=S[:], start=first, stop=last)
                if last:
                    sl = accT[:, t * P : (t + 1) * P]
                    if not written[t]:
                        nc.scalar.activation(sl, psum[:], AF.Copy)
                        written[t] = True
                    else:
                        nc.vector.tensor_tensor(sl, psum[:], sl, ALU.add)
                ci += 1
                gchunk += 1
    for t in range(plan.n_tiles):
        if not written[t]:
            nc.vector.memset(accT[:, t * P : (t + 1) * P], 0.0)


def _spmm_N(nc, sb, gpool, spool, psp, plan, tables_per_half, idx_tiles, w_col, rel_col,
            iota, acc, n_tbl, gtag, pstag):
    """Non-transposed spmm with packed tables: per chunk one matmul
    psum[dst, n_tbl*128] = S.T-weighted scatter of all tables; accumulate into
    acc [128, n_tiles, n_tbl*128]."""
    W = n_tbl * P
    written = [False] * plan.n_tiles
    gchunk = 0
    for h in range(plan.n_halves):
        nch_h = plan.half_nchunks[h]
        if nch_h == 0:
            continue
        Lh = nch_h * P
        sched = plan.sched[h]
        ci = 0
        psum = None
        for b0 in range(0, Lh, IDX_BATCH):
            nidx = min(IDX_BATCH, Lh - b0)
            nch = nidx // P
            gt = gpool.tile([P, n_tbl, IDX_BATCH // P, P], F32, tag=gtag)
            for ti in range(n_tbl):
                nc.gpsimd.dma_gather(
                    gt[:, ti, :nch, :], tables_per_half[h][ti],
                    idx_tiles[h][:, b0 // 16 : (b0 + nidx) // 16], nidx, nidx, P,
                )
            for k in range(nch):
                t, first, last = sched[ci]
                if first:
                    psum = psp.tile([P, W], F32, tag=pstag)
                S = spool.tile([P, P], F32, tag="S")
                nc.vector.tensor_scalar(
                    S[:], iota[:], rel_col[:, gchunk : gchunk + 1],
                    w_col[:, gchunk : gchunk + 1], ALU.is_equal, ALU.mult,
                )
                nc.tensor.matmul(psum[:], lhsT=S[:], rhs=gt[:, :, k, :], start=first, stop=last)
                if last:
                    sl = acc[:, t, :]
                    if not written[t]:
                        nc.scalar.activation(sl, psum[:], AF.Copy)
                        written[t] = True
                    else:
                        nc.vector.tensor_tensor(sl, psum[:], sl, ALU.add)
                ci += 1
                gchunk += 1
    for t in range(plan.n_tiles):
        if not written[t]:
            nc.vector.memset(acc[:, t, :], 0.0)


def _gcn_second(nc, psp, accT, w_t, b_row, ones, outb, n_tiles):
    """outb[:, t, :] = relu(accT_t.T @ W + b)."""
    for t in range(n_tiles):
        ps = psp.tile([P, P], F32, tag="ps2")
        nc.tensor.matmul(ps[:], lhsT=accT[:, t * P : (t + 1) * P], rhs=w_t[:], start=True, stop=False)
        nc.tensor.matmul(ps[:], lhsT=ones[:1, :], rhs=b_row[:1, :], start=False, stop=True)
        nc.scalar.activation(outb[:, t, :], ps[:], AF.Relu)


def _l2norm_scale(nc, sb, src_ap, width, out_writes):
    """out = src / (||src||_2 + EPS) along free axis; out_writes = [(dst_ap, src_slice_ap)]."""
    sq = sb.tile([P, width], F32, tag="sq")
    ss = sb.tile([P, 1], F32, tag="ss")
    nc.scalar.activation(sq[:], src_ap, AF.Square, accum_out=ss[:])
    nrm = sb.tile([P, 1], F32, tag="nrm")
    nc.scalar.activation(nrm[:], ss[:], AF.Sqrt)
    nc.vector.tensor_scalar_add(nrm[:], nrm[:], EPS)
    rn = sb.tile([P, 1], F32, tag="rn")
    nc.vector.reciprocal(rn[:], nrm[:])
    for dst_ap, s_ap in out_writes:
        nc.vector.tensor_scalar_mul(dst_ap, s_ap, rn[:])


# ---------------------------------------------------------------------------
# Full program
# ---------------------------------------------------------------------------

def _dump(nc, tc, src_dram, dst, n_rows, width):
    with tc.tile_pool(name="dump", bufs=2) as dp:
        for t in range(n_rows // P):
            tl = dp.tile([P, width], F32, tag="d")
            nc.sync.dma_start(tl[:], src_dram[t * P : (t + 1) * P, :])
            nc.sync.dma_start(dst[t * P : (t + 1) * P, :], tl[:])


def build_program(p11, p22, p01, p02, stop_after="full"):
    _ORDER = ["A", "AG1", "B1", "AG3", "B2", "full"]

    def want(p):
        return _ORDER.index(stop_after) >= _ORDER.index(p)
    nc = bacc.Bacc("TRN2", num_devices=NC)

    # external inputs (per-core)
    x1T = nc.dram_tensor("x1T", [D, S1P], F32, kind="ExternalInput")
    x2T = nc.dram_tensor("x2T", [D, S2P], F32, kind="ExternalInput")
    wembp = nc.dram_tensor("wembp", [NC * S2P, DW], F32, kind="ExternalInput")
    w1a = nc.dram_tensor("w1a", [D, O], F32, kind="ExternalInput")
    w1b = nc.dram_tensor("w1b", [O, O], F32, kind="ExternalInput")
    w2a = nc.dram_tensor("w2a", [D, O], F32, kind="ExternalInput")
    w2b = nc.dram_tensor("w2b", [O, O], F32, kind="ExternalInput")
    biases = nc.dram_tensor("biases", [4, O], F32, kind="ExternalInput")  # b1a,b1b,b2a,b2b
    iota_in = nc.dram_tensor("iota", [P, P], F32, kind="ExternalInput")
    ones_in = nc.dram_tensor("ones", [1, P], F32, kind="ExternalInput")

    def edge_inputs(name, plan):
        idx = [
            nc.dram_tensor(f"{name}_idx{h}", list(plan.idx_wrapped[0][h].shape), I16,
                           kind="ExternalInput")
            for h in range(plan.n_halves)
        ]
        wv = nc.dram_tensor(f"{name}_w", [P, plan.total_chunks], F32, kind="ExternalInput")
        rel = nc.dram_tensor(f"{name}_rel", [P, plan.total_chunks], F32, kind="ExternalInput")
        return idx, wv, rel

    e11_idx, e11_w, e11_rel = edge_inputs("e11", p11)
    e22_idx, e22_w, e22_rel = edge_inputs("e22", p22)
    e01_idx, e01_w, e01_rel = edge_inputs("e01", p01)
    e02_idx, e02_w, e02_rel = edge_inputs("e02", p02)

    # internal node tables (padded coords), full copies via AllGather
    l11_loc = nc.dram_tensor("l11_loc", [S1P, O], F32)
    l12_loc = nc.dram_tensor("l12_loc", [S2P, O], F32)
    l21_loc = nc.dram_tensor("l21_loc", [S1P, O], F32)
    l22_loc = nc.dram_tensor("l22_loc", [S2P, O], F32)
    l11_full = nc.dram_tensor("l11_full", [NC * S1P, O], F32, addr_space="Shared")
    l12_full = nc.dram_tensor("l12_full", [NC * S2P, O], F32, addr_space="Shared")
    l21_full = nc.dram_tensor("l21_full", [NC * S1P, O], F32, addr_space="Shared")
    l22_full = nc.dram_tensor("l22_full", [NC * S2P, O], F32, addr_space="Shared")

    # outputs
    doc_loc = nc.dram_tensor("doc_local", [S0P, 2 * O + DW], F32, kind="ExternalOutput")
    docsvd_loc = nc.dram_tensor("docsvd_local", [S0P, 2 * O + DW], F32, kind="ExternalOutput")

    rg = [list(range(NC))]

    class _StopBuild(Exception):
        pass

    dbg = {}
    if stop_after != "full":
        dbg["l11"] = nc.dram_tensor("dbg_l11", [S1P, O], F32, kind="ExternalOutput")
        dbg["l12"] = nc.dram_tensor("dbg_l12", [S2P, O], F32, kind="ExternalOutput")
    if stop_after in ("AG1", "B1", "AG3", "B2"):
        dbg["l11f"] = nc.dram_tensor("dbg_l11f", [NC * S1P, O], F32, kind="ExternalOutput")
    if stop_after in ("B1", "AG3", "B2"):
        dbg["l21"] = nc.dram_tensor("dbg_l21", [S1P, O], F32, kind="ExternalOutput")
    if stop_after == "B2":
        dbg["l22"] = nc.dram_tensor("dbg_l22", [S2P, O], F32, kind="ExternalOutput")

    def emit_dumps(tc):
        if "l11" in dbg:
            _dump(nc, tc, l11_loc[:], dbg["l11"][:], S1P, O)
            _dump(nc, tc, l12_loc[:], dbg["l12"][:], S2P, O)
        if "l11f" in dbg:
            _dump(nc, tc, l11_full[:], dbg["l11f"][:], NC * S1P, O)
        if "l21" in dbg:
            _dump(nc, tc, l21_loc[:], dbg["l21"][:], S1P, O)
        if "l22" in dbg:
            _dump(nc, tc, l22_loc[:], dbg["l22"][:], S2P, O)

    with TileContext(nc) as tc:
      try:
        with tc.tile_pool(name="const", bufs=1) as cp:
            iota = cp.tile([P, P], F32)
            ones = cp.tile([1, P], F32)
            nc.sync.dma_start(iota[:], iota_in[:])
            nc.sync.dma_start(ones[:], ones_in[:])
            w1a0 = cp.tile([P, O], F32); nc.sync.dma_start(w1a0[:], w1a[0:P, :])
            w1a1 = cp.tile([P, O], F32); nc.sync.dma_start(w1a1[:], w1a[P:D, :])
            w2a0 = cp.tile([P, O], F32); nc.sync.dma_start(w2a0[:], w2a[0:P, :])
            w2a1 = cp.tile([P, O], F32); nc.sync.dma_start(w2a1[:], w2a[P:D, :])
            w1bt = cp.tile([O, O], F32); nc.sync.dma_start(w1bt[:], w1b[:])
            w2bt = cp.tile([O, O], F32); nc.sync.dma_start(w2bt[:], w2b[:])
            btiles = []
            for i in range(4):
                bt = cp.tile([1, O], F32, tag=f"b{i}")
                nc.sync.dma_start(bt[:], biases[i : i + 1, :])
                btiles.append(bt)
            b1a, b1b, b2a, b2b = (bt[:] for bt in btiles)

            # A/B-phase PSUM pools, explicitly closed before phase C (PSUM is 8 banks)
            ab_pools = ExitStack()
            psp = ab_pools.enter_context(tc.tile_pool(name="psum", bufs=2, space="PSUM"))
            pspw = ab_pools.enter_context(tc.tile_pool(name="psumW", bufs=2, space="PSUM"))

            # ---- phase A: first (identity) GCN layers, sharded rows ----
            with tc.tile_pool(name="phA", bufs=1) as sa:
                outb1 = sa.tile([P, T1, O], F32)
                _dense_layer(nc, sa, psp, x1T, T1, w1a0, w1a1, b1a, ones, outb1)
                nc.sync.dma_start(l11_loc[:].rearrange("(t p) f -> p t f", p=P), outb1[:])
                outb2 = sa.tile([P, T2, O], F32)
                _dense_layer(nc, sa, psp, x2T, T2, w2a0, w2a1, b2a, ones, outb2)
                nc.sync.dma_start(l12_loc[:].rearrange("(t p) f -> p t f", p=P), outb2[:])

            if not want("AG1"):
                raise _StopBuild()
            nc.gpsimd.collective_compute(
                "AllGather", ALU.bypass, replica_groups=rg,
                ins=[l11_loc[:]], outs=[l11_full[:]])
            nc.gpsimd.collective_compute(
                "AllGather", ALU.bypass, replica_groups=rg,
                ins=[l12_loc[:]], outs=[l12_full[:]])

            l11_halves = [l11_full[0:HALF1, :], l11_full[HALF1 : 2 * HALF1, :]]
            l21_halves = [l21_full[0:HALF1, :], l21_full[HALF1 : 2 * HALF1, :]]

            # ---- phase B1: spmm(e11, l1_1) -> @W1b -> l2_1 ----
            if not want("B1"):
                raise _StopBuild()
            with (
                tc.tile_pool(name="phB1", bufs=1) as sb1,
                tc.tile_pool(name="gpoolB1", bufs=3) as gp1,
                tc.tile_pool(name="spoolB1", bufs=4) as sp1,
            ):
                idx_t = []
                for h in range(p11.n_halves):
                    it = sb1.tile(list(p11.idx_wrapped[0][h].shape), I16, tag=f"idx{h}")
                    nc.sync.dma_start(it[:], e11_idx[h][:])
                    idx_t.append(it)
                wct = sb1.tile([P, p11.total_chunks], F32, tag="wc")
                relt = sb1.tile([P, p11.total_chunks], F32, tag="rc")
                nc.sync.dma_start(wct[:], e11_w[:])
                nc.sync.dma_start(relt[:], e11_rel[:])
                accT = sb1.tile([P, S1P], F32, tag="accT")
                _spmm_T(nc, sb1, gp1, sp1, psp, p11, l11_halves, idx_t, wct, relt, iota, accT)
                outb = sb1.tile([P, T1, O], F32, tag="outb")
                _gcn_second(nc, pspw, accT, w1bt, b1b, ones, outb, T1)
                nc.sync.dma_start(l21_loc[:].rearrange("(t p) f -> p t f", p=P), outb[:])

            if not want("AG3"):
                raise _StopBuild()
            nc.gpsimd.collective_compute(
                "AllGather", ALU.bypass, replica_groups=rg,
                ins=[l21_loc[:]], outs=[l21_full[:]])

            # ---- phase B2: spmm(e22, l1_2) -> @W2b -> l2_2 ----
            if not want("B2"):
                raise _StopBuild()
            with (
                tc.tile_pool(name="phB2", bufs=1) as sb2,
                tc.tile_pool(name="gpoolB2", bufs=3) as gp2,
                tc.tile_pool(name="spoolB2", bufs=4) as sp2,
            ):
                idx_t = []
                for h in range(p22.n_halves):
                    it = sb2.tile(list(p22.idx_wrapped[0][h].shape), I16, tag=f"idx{h}")
                    nc.sync.dma_start(it[:], e22_idx[h][:])
                    idx_t.append(it)
                wct = sb2.tile([P, p22.total_chunks], F32, tag="wc")
                relt = sb2.tile([P, p22.total_chunks], F32, tag="rc")
                nc.sync.dma_start(wct[:], e22_w[:])
                nc.sync.dma_start(relt[:], e22_rel[:])
                accT = sb2.tile([P, S2P], F32, tag="accT")
                _spmm_T(nc, sb2, gp2, sp2, psp, p22, [l12_full[:]], idx_t, wct, relt, iota, accT)
                outb = sb2.tile([P, T2, O], F32, tag="outb")
                _gcn_second(nc, pspw, accT, w2bt, b2b, ones, outb, T2)
                nc.sync.dma_start(l22_loc[:].rearrange("(t p) f -> p t f", p=P), outb[:])

            nc.gpsimd.collective_compute(
                "AllGather", ALU.bypass, replica_groups=rg,
                ins=[l22_loc[:]], outs=[l22_full[:]])

            ab_pools.close()

            # ---- phase C: doc aggregation ----
            if stop_after != "full":
                raise _StopBuild()
            with (
                tc.tile_pool(name="phC", bufs=1) as sc,
                tc.tile_pool(name="psC", bufs=2, space="PSUM") as psc,
            ):
                acc01 = sc.tile([P, T0, 2 * O], F32, tag="acc01")
                acc02 = sc.tile([P, T0, 3 * O], F32, tag="acc02")

                with (
                    tc.tile_pool(name="gpoolC1", bufs=2) as gpc1,
                    tc.tile_pool(name="spoolC1", bufs=4) as spc1,
                ):
                    idx_t = []
                    for h in range(p01.n_halves):
                        it = sc.tile(list(p01.idx_wrapped[0][h].shape), I16, tag=f"i01_{h}")
                        nc.sync.dma_start(it[:], e01_idx[h][:])
                        idx_t.append(it)
                    wct = sc.tile([P, p01.total_chunks], F32, tag="w01")
                    relt = sc.tile([P, p01.total_chunks], F32, tag="r01")
                    nc.sync.dma_start(wct[:], e01_w[:])
                    nc.sync.dma_start(relt[:], e01_rel[:])
                    tables = [[l21_halves[h], l11_halves[h]] for h in range(p01.n_halves)]
                    _spmm_N(nc, sc, gpc1, spc1, psc, p01, tables, idx_t,
                            wct, relt, iota, acc01[:], 2, "g01", "ps01")

                with (
                    tc.tile_pool(name="gpoolC2", bufs=2) as gpc2,
                    tc.tile_pool(name="spoolC2", bufs=4) as spc2,
                ):
                    idx_t = []
                    for h in range(p02.n_halves):
                        it = sc.tile(list(p02.idx_wrapped[0][h].shape), I16, tag=f"i02_{h}")
                        nc.sync.dma_start(it[:], e02_idx[h][:])
                        idx_t.append(it)
                    wct = sc.tile([P, p02.total_chunks], F32, tag="w02")
                    relt = sc.tile([P, p02.total_chunks], F32, tag="r02")
                    nc.sync.dma_start(wct[:], e02_w[:])
                    nc.sync.dma_start(relt[:], e02_rel[:])
                    # packed tables: [l2_2, wemb, l1_2] so r1 = cols 0:256, r1s = cols 256:384 + 128:256
                    tables = [[l22_full[:], wembp[:], l12_full[:]]]
                    _spmm_N(nc, sc, gpc2, spc2, psc, p02, tables, idx_t,
                            wct, relt, iota, acc02[:], 3, "g02", "ps02")

                docb = sc.tile([P, T0, 2 * O + DW], F32, tag="docb")
                docsb = sc.tile([P, T0, 2 * O + DW], F32, tag="docsb")
                for t in range(T0):
                    # doc: [l2norm(r0) | l2norm(r1)]; r0 = acc01[:,:,0:128], r1 = acc02[:,:,0:256]
                    _l2norm_scale(nc, sc, acc01[:, t, 0:O], O,
                                  [(docb[:, t, 0:O], acc01[:, t, 0:O])])
                    _l2norm_scale(nc, sc, acc02[:, t, 0 : 2 * O], 2 * O,
                                  [(docb[:, t, O : 3 * O], acc02[:, t, 0 : 2 * O])])
                    # doc_svd: r0s = acc01 cols 128:256; r1s = [l1_2 | wemb] = acc02 cols [256:384, 128:256]
                    _l2norm_scale(nc, sc, acc01[:, t, O : 2 * O], O,
                                  [(docsb[:, t, 0:O], acc01[:, t, O : 2 * O])])
                    sq = sc.tile([P, 2 * O], F32, tag="sqs")
                    ss = sc.tile([P, 1], F32, tag="sss")
                    # norm over r1s = cols 128:384 of acc02 (wemb + l1_2, order-independent)
                    nc.scalar.activation(sq[:], acc02[:, t, O : 3 * O], AF.Square, accum_out=ss[:])
                    nrm = sc.tile([P, 1], F32, tag="nrms")
                    nc.scalar.activation(nrm[:], ss[:], AF.Sqrt)
                    nc.vector.tensor_scalar_add(nrm[:], nrm[:], EPS)
                    rn = sc.tile([P, 1], F32, tag="rns")
                    nc.vector.reciprocal(rn[:], nrm[:])
                    nc.vector.tensor_scalar_mul(docsb[:, t, O : 2 * O], acc02[:, t, 2 * O : 3 * O], rn[:])
                    nc.vector.tensor_scalar_mul(docsb[:, t, 2 * O : 2 * O + DW], acc02[:, t, O : 2 * O], rn[:])
                nc.sync.dma_start(doc_loc[:].rearrange("(t p) f -> p t f", p=P), docb[:])
                nc.sync.dma_start(docsvd_loc[:].rearrange("(t p) f -> p t f", p=P), docsb[:])

      except _StopBuild:
        try:
            ab_pools.close()
        except Exception:
            pass
      if stop_after != "full":
        emit_dumps(tc)
    nc.compile()
    return nc


# ---------------------------------------------------------------------------
# Host wrapper
# ---------------------------------------------------------------------------

_CACHE = {}


def _prep(inputs):
    x1 = np.asarray(inputs["x1"], np.float32)
    x2 = np.asarray(inputs["x2"], np.float32)
    wemb = np.asarray(inputs["word_emb"], np.float32)

    p11 = EdgePlan(inputs["e11_src"], inputs["e11_dst"], inputs["e11_w"],
                   S1, S1P, S1, S1P, HALF1)
    p22 = EdgePlan(inputs["e22_src"], inputs["e22_dst"], inputs["e22_w"],
                   S2, S2P, S2, S2P, None)
    p01 = EdgePlan(inputs["e01_src"], inputs["e01_dst"], inputs["e01_w"],
                   S0, S0P, S1, S1P, HALF1)
    p02 = EdgePlan(inputs["e02_src"], inputs["e02_dst"], inputs["e02_w"],
                   S0, S0P, S2, S2P, None)

    wembp = np.zeros((NC * S2P, DW), np.float32)
    for c in range(NC):
        wembp[c * S2P : c * S2P + S2] = wemb[c * S2 : (c + 1) * S2]

    iota = np.tile(np.arange(P, dtype=np.float32), (P, 1))
    ones = np.ones((1, P), np.float32)
    biases = np.stack([
        np.asarray(inputs["b1a"], np.float32), np.asarray(inputs["b1b"], np.float32),
        np.asarray(inputs["b2a"], np.float32), np.asarray(inputs["b2b"], np.float32),
    ])

    in_maps = []
    for c in range(NC):
        x1T = np.zeros((D, S1P), np.float32)
        x1T[:, :S1] = x1[c * S1 : (c + 1) * S1].T
        x2T = np.zeros((D, S2P), np.float32)
        x2T[:, :S2] = x2[c * S2 : (c + 1) * S2].T
        m = {
            "x1T": x1T, "x2T": x2T, "wembp": wembp,
            "w1a": np.asarray(inputs["W1a"], np.float32),
            "w1b": np.asarray(inputs["W1b"], np.float32),
            "w2a": np.asarray(inputs["W2a"], np.float32),
            "w2b": np.asarray(inputs["W2b"], np.float32),
            "biases": biases, "iota": iota, "ones": ones,
        }
        for name, plan in (("e11", p11), ("e22", p22), ("e01", p01), ("e02", p02)):
            for h in range(plan.n_halves):
                m[f"{name}_idx{h}"] = plan.idx_wrapped[c][h]
            m[f"{name}_w"] = np.ascontiguousarray(plan.w_wrapped[c])
            m[f"{name}_rel"] = np.ascontiguousarray(plan.rel_wrapped[c])
        in_maps.append(m)
    return (p11, p22, p01, p02), in_maps


def get_compiled(inputs):
    """Build (or fetch cached) program + in_maps for these inputs."""
    plans, in_maps = _prep(inputs)
    key = tuple(p.total_chunks for p in plans) + tuple(
        tuple(p.n_chunks.reshape(-1).tolist()) for p in plans
    )
    if key not in _CACHE:
        _CACHE[key] = build_program(*plans)
    return _CACHE[key], in_maps


def kernel(**inputs):
    nc, in_maps = get_compiled(inputs)
    res = run_bass_kernel_spmd(nc, in_maps, core_ids=list(range(NC)), trace=False)
    doc = np.concatenate([res.results[c]["doc_local"][:S0] for c in range(NC)], axis=0)
    dsvd = np.concatenate([res.results[c]["docsvd_local"][:S0] for c in range(NC)], axis=0)
    return (doc[:N0], dsvd[:N0])
